# revision 27
# baseline (speedup 1.0000x reference)
"""Trainium2 Bass kernel for nn_AttentionLayer_77558519431766.

Math: the reference computes softmax over a size-1 axis, which is
identically 1.0, so the attention MLP is dead code and

    out[b, e] = sum_{i<j} x[b,i,e] * x[b,j,e]
              = 0.5 * ((sum_f x[b,f,e])^2 - sum_f x[b,f,e]^2)

Implementation (per 128-sample chunk, layout [128b, f*64+e]):
  1. ACT casts x to bf16.
  2. PE transposes each [128b, 128(f2,e)] block into PSUM, so pairs of
     f-rows land on partitions.
  3. DVE copies the transposed blocks back to SBUF (for s), ACT squares
     them into SBUF (for q).
  4. PE runs two matmul accumulation chains against a stacked-identity
     mask [128,64] (row (f2,e) is one-hot at e), yielding
     s = sum_f x and q = sum_f x^2 as [128b, 64e] in PSUM.
  5. res = 0.5*s^2 - 0.5*q, DMA out.

Sharding: pure data parallelism, batch 2048 -> 8 shards of 256.
"""

import numpy as np

try:
    import concourse.bass as bass  # noqa: F401
except ImportError:  # pragma: no cover
    import sys

    sys.path.insert(0, "/opt/trn_rl_repo")

_B, _F, _E = 2048, 50, 64
_NCORES = 8
_BS = _B // _NCORES  # 256 rows per core
_ROW = _F * _E  # 3200 floats per row
_P = 128  # SBUF partitions
_NBLK = _ROW // _P  # 25 transpose blocks per chunk


def _make_tc_class():
    """TileContext with a slim kernel tail.

    Stock TileContext ends with drain -> full all-engine barrier ->
    semaphore clear -> second full barrier (~6-8us of EVSEM butterfly).
    The Bass preamble already dma_reset+sem_clears the entire kernel
    semaphore range at the start of every execution, so the tail clear
    and second barrier are redundant for a single-TileContext kernel.
    Keep the global-clock drain (output DMA completion) plus one cheap
    sequencer-level barrier.
    """
    from concourse.tile import TileContext
    from concourse.vector_clock import ScopedClock

    class SlimTailTileContext(TileContext):
        def _drain_and_barrier(self, tick_clock, wait_clock):
            drain_inst = self.nc.sync.drain()
            wait_clock.add_sem_waits(
                drain_inst.ins, ScopedClock({None: tick_clock.global_clock})
            )
            self.nc.all_engine_barrier(sem_only=True)
            popped = self.nc._tile_sem_poison_stack.pop()
            assert popped is self._sem_poison

    return SlimTailTileContext


def _build():
    import concourse.bacc as bacc
    import concourse.mybir as mybir

    TileContext = _make_tc_class()

    f32 = mybir.dt.float32
    bf16 = mybir.dt.bfloat16
    SQ = mybir.ActivationFunctionType.Square
    CP = mybir.ActivationFunctionType.Copy
    ALU = mybir.AluOpType
    HALF_SQRT = float(np.float32(np.sqrt(0.5)))

    i32 = mybir.dt.int32

    nc = bacc.Bacc()
    x = nc.declare_dram_parameter("inputs", [_BS, _ROW], f32, isOutput=False)
    out = nc.declare_dram_parameter("out", [_BS, _E], f32, isOutput=True)

    n_chunks = _BS // _P  # 2
    halves = [(0, _NBLK)]  # full-width DMA keeps 12.8KB/row packets (345 GB/s)
    groups = [7, 6, 6, 6]  # transpose blocks per PSUM tile (<= 1 bank bf16)

    with TileContext(nc) as tc:
        with (
            tc.tile_pool(name="consts", bufs=1) as cpool,
            tc.tile_pool(name="x", bufs=4) as xpool,
            tc.tile_pool(name="xb", bufs=4) as xbpool,
            tc.tile_pool(name="xT", bufs=4) as xtpool,
            tc.tile_pool(name="xsq", bufs=4) as sqpool,
            tc.tile_pool(name="pt", bufs=3, space="PSUM") as ptpool,
            tc.tile_pool(name="acc", bufs=2, space="PSUM") as accpool,
            tc.tile_pool(name="small", bufs=2) as spool,
        ):
            # Warm op: forces the ACT function-table load off the critical
            # path (it otherwise lands right before the first Square, after
            # a cross-engine wait).
            warm = spool.tile([_P, 1], f32, tag="warm")
            nc.gpsimd.memset(warm[:], 0.0)
            nc.scalar.activation(warm[:], warm[:], SQ)

            # Constants built on-chip (a DMA for these queues behind the
            # input packets and stalls the first transposes by multiple us).
            # iota with channel_multiplier=-1 gives v[p,j] = j - p, so
            # identity = (v == 0); the stacked mask [128,64] has ones where
            # j - p is 0 or -64.
            iot_i = cpool.tile([_P, _P], i32, tag="iot_i")
            iot_m = cpool.tile([_P, _E], i32, tag="iot_m")
            ident = cpool.tile([_P, _P], bf16, tag="ident")
            mask = cpool.tile([_P, _E], bf16, tag="mask")
            mask_b = cpool.tile([_P, _E], bf16, tag="mask_b")
            nc.gpsimd.iota(iot_i[:], pattern=[[1, _P]], base=0, channel_multiplier=-1)
            nc.gpsimd.iota(iot_m[:], pattern=[[1, _E]], base=0, channel_multiplier=-1)
            nc.vector.tensor_scalar(
                ident[:], iot_i[:], 0, None, op0=ALU.is_equal
            )
            nc.vector.tensor_scalar(
                mask[:], iot_m[:], 0, None, op0=ALU.is_equal
            )
            nc.vector.tensor_scalar(
                mask_b[:], iot_m[:], -_E, None, op0=ALU.is_equal
            )
            nc.vector.tensor_add(mask[:], mask[:], mask_b[:])

            for c in range(n_chunks):
                rows = slice(c * _P, (c + 1) * _P)
                # separate banks: a start=True matmul clears its whole bank,
                # so the two accumulation chains must not share one
                s_t = accpool.tile([_P, _E], f32, tag="s")
                q_t = accpool.tile([_P, _E], f32, tag="q")
                s_ps = s_t[:]
                q_ps = q_t[:]
                for blk0, nblk in halves:
                    cols = slice(blk0 * _P, (blk0 + nblk) * _P)
                    n = nblk * _P
                    xt = xpool.tile([_P, n], f32, tag="x")
                    nc.sync.dma_start(out=xt[:], in_=x[rows, cols])
                    xbt = xbpool.tile([_P, n], bf16, tag="xb")

                    xT = xtpool.tile([_P, n], bf16, tag="xT")
                    xsq = sqpool.tile([_P, n], bf16, tag="xsq")
                    g0 = 0
                    for gn in groups:
                        gcols = slice(g0 * _P, (g0 + gn) * _P)
                        # per-group cast: lets the tail chunk pipeline at
                        # group granularity (gpsimd CAST measured 4x slower
                        # than DVE's 2x mode, so this stays on DVE)
                        nc.vector.tensor_copy(xbt[:, gcols], xt[:, gcols])
                        pt = ptpool.tile([_P, groups[0] * _P], bf16, tag="pt")
                        for j in range(gn):
                            k = g0 + j
                            nc.tensor.transpose(
                                pt[:, j * _P : (j + 1) * _P],
                                xbt[:, k * _P : (k + 1) * _P],
                                ident[:],
                            )
                        nc.vector.tensor_copy(xT[:, gcols], pt[:, : gn * _P])
                        nc.scalar.activation(xsq[:, gcols], pt[:, : gn * _P], SQ)
                        g0 += gn
                    for k in range(nblk):
                        kk = blk0 + k
                        bcols = slice(k * _P, (k + 1) * _P)
                        nc.tensor.matmul(
                            s_ps,
                            xT[:, bcols],
                            mask[:],
                            start=(kk == 0),
                            stop=(kk == _NBLK - 1),
                        )
                        nc.tensor.matmul(
                            q_ps,
                            xsq[:, bcols],
                            mask[:],
                            start=(kk == 0),
                            stop=(kk == _NBLK - 1),
                        )

                # res = 0.5*s^2 - 0.5*q
                s2h = spool.tile([_P, _E], f32, tag="s2h")
                res = spool.tile([_P, _E], f32, tag="res")
                nc.scalar.activation(s2h[:], s_ps, SQ, scale=HALF_SQRT)
                nc.vector.scalar_tensor_tensor(
                    res[:], q_ps, -0.5, s2h[:], op0=ALU.mult, op1=ALU.add
                )
                nc.sync.dma_start(out=out[rows, :], in_=res[:])
    nc.compile()
    return nc


_WALRUS_EXTRA = ["--max-sem-num=32"]


def _patch_walrus():
    """Cap walrus's semaphore allocation: the NEFF postamble zeroes every
    allocated semaphore one event-sem op at a time (spaced to dodge the
    event-accel erratum), so unused semaphores cost ~150ns each at the
    kernel tail."""
    from concourse import bass_utils

    if getattr(bass_utils, "_walrus_patched", False):
        return
    real_run = bass_utils.run_command

    def run2(cmd, **kw):
        if cmd and "walrus_driver" in str(cmd[0]):
            cmd = list(cmd) + _WALRUS_EXTRA
        return real_run(cmd, **kw)

    bass_utils.run_command = run2
    bass_utils._walrus_patched = True


def _run(in_maps, **kwargs):
    from concourse.bass_utils import run_bass_kernel_spmd

    _patch_walrus()
    nc = _build()
    return run_bass_kernel_spmd(nc, in_maps, core_ids=list(range(_NCORES)), **kwargs)


def _shard(inputs: np.ndarray):
    x = np.ascontiguousarray(
        np.asarray(inputs, dtype=np.float32).reshape(_B, _ROW)
    )
    return [
        {"inputs": np.ascontiguousarray(x[i * _BS : (i + 1) * _BS])}
        for i in range(_NCORES)
    ]


def kernel(
    inputs: np.ndarray,
    weight_attention: np.ndarray = None,
    weight_projection: np.ndarray = None,
    weight_bias: np.ndarray = None,
) -> np.ndarray:
    # weights are dead code (softmax over a size-1 axis == 1.0)
    res = _run(_shard(inputs))
    return np.concatenate([r["out"] for r in res.results], axis=0)


# revision 32
# speedup vs baseline: 1.0266x; 1.0266x over previous
"""Trainium2 Bass kernel for nn_AttentionLayer_77558519431766.

Math: the reference computes softmax over a size-1 axis, which is
identically 1.0, so the attention MLP is dead code and

    out[b, e] = sum_{i<j} x[b,i,e] * x[b,j,e]
              = 0.5 * ((sum_f x[b,f,e])^2 - sum_f x[b,f,e]^2)

Implementation (per 128-sample chunk, layout [128b, f*64+e]):
  1. ACT casts x to bf16.
  2. PE transposes each [128b, 128(f2,e)] block into PSUM, so pairs of
     f-rows land on partitions.
  3. DVE copies the transposed blocks back to SBUF (for s), ACT squares
     them into SBUF (for q).
  4. PE runs two matmul accumulation chains against a stacked-identity
     mask [128,64] (row (f2,e) is one-hot at e), yielding
     s = sum_f x and q = sum_f x^2 as [128b, 64e] in PSUM.
  5. res = 0.5*s^2 - 0.5*q, DMA out.

Sharding: pure data parallelism, batch 2048 -> 8 shards of 256.
"""

import numpy as np

try:
    import concourse.bass as bass  # noqa: F401
except ImportError:  # pragma: no cover
    import sys

    sys.path.insert(0, "/opt/trn_rl_repo")

_B, _F, _E = 2048, 50, 64
_NCORES = 8
_BS = _B // _NCORES  # 256 rows per core
_ROW = _F * _E  # 3200 floats per row
_P = 128  # SBUF partitions
_NBLK = _ROW // _P  # 25 transpose blocks per chunk


def _make_tc_class():
    """TileContext with a slim kernel tail.

    Stock TileContext ends with drain -> full all-engine barrier ->
    semaphore clear -> second full barrier (~6-8us of EVSEM butterfly).
    The Bass preamble already dma_reset+sem_clears the entire kernel
    semaphore range at the start of every execution, so the tail clear
    and second barrier are redundant for a single-TileContext kernel.
    Keep the global-clock drain (output DMA completion) plus one cheap
    sequencer-level barrier.
    """
    from concourse.tile import TileContext
    from concourse.vector_clock import ScopedClock

    class SlimTailTileContext(TileContext):
        def _drain_and_barrier(self, tick_clock, wait_clock):
            drain_inst = self.nc.sync.drain()
            wait_clock.add_sem_waits(
                drain_inst.ins, ScopedClock({None: tick_clock.global_clock})
            )
            self.nc.all_engine_barrier(sem_only=True)
            popped = self.nc._tile_sem_poison_stack.pop()
            assert popped is self._sem_poison

    return SlimTailTileContext


def _build():
    import concourse.bacc as bacc
    import concourse.mybir as mybir

    TileContext = _make_tc_class()

    f32 = mybir.dt.float32
    bf16 = mybir.dt.bfloat16
    SQ = mybir.ActivationFunctionType.Square
    CP = mybir.ActivationFunctionType.Copy
    ALU = mybir.AluOpType
    HALF_SQRT = float(np.float32(np.sqrt(0.5)))

    i32 = mybir.dt.int32

    nc = bacc.Bacc()
    x = nc.declare_dram_parameter("inputs", [_BS, _ROW], f32, isOutput=False)
    out = nc.declare_dram_parameter("out", [_BS, _E], f32, isOutput=True)

    n_chunks = _BS // _P  # 2
    halves = [(0, _NBLK)]  # full-width DMA keeps 12.8KB/row packets (345 GB/s)
    groups = [7, 6, 6, 6]  # transpose blocks per PSUM tile (<= 1 bank bf16)

    with TileContext(nc) as tc:
        with (
            tc.tile_pool(name="consts", bufs=1) as cpool,
            tc.tile_pool(name="x", bufs=4) as xpool,
            tc.tile_pool(name="xb", bufs=4) as xbpool,
            tc.tile_pool(name="xT", bufs=4) as xtpool,
            tc.tile_pool(name="xsq", bufs=4) as sqpool,
            tc.tile_pool(name="pt", bufs=3, space="PSUM") as ptpool,
            tc.tile_pool(name="acc", bufs=2, space="PSUM") as accpool,
            tc.tile_pool(name="small", bufs=2) as spool,
        ):
            # Warm op: forces the ACT function-table load off the critical
            # path (it otherwise lands right before the first Square, after
            # a cross-engine wait).
            warm = spool.tile([_P, 1], f32, tag="warm")
            nc.gpsimd.memset(warm[:], 0.0)
            nc.scalar.activation(warm[:], warm[:], SQ)

            # Constants built on-chip (a DMA for these queues behind the
            # input packets and stalls the first transposes by multiple us).
            # iota with channel_multiplier=-1 gives v[p,j] = j - p, so
            # identity = (v == 0); the stacked mask [128,64] has ones where
            # j - p is 0 or -64.
            iot_i = cpool.tile([_P, _P], i32, tag="iot_i")
            iot_m = cpool.tile([_P, _E], i32, tag="iot_m")
            ident = cpool.tile([_P, _P], bf16, tag="ident")
            mask = cpool.tile([_P, _E], bf16, tag="mask")
            mask_b = cpool.tile([_P, _E], bf16, tag="mask_b")
            nc.gpsimd.iota(iot_i[:], pattern=[[1, _P]], base=0, channel_multiplier=-1)
            nc.gpsimd.iota(iot_m[:], pattern=[[1, _E]], base=0, channel_multiplier=-1)
            nc.vector.tensor_scalar(
                ident[:], iot_i[:], 0, None, op0=ALU.is_equal
            )
            nc.vector.tensor_scalar(
                mask[:], iot_m[:], 0, None, op0=ALU.is_equal
            )
            nc.vector.tensor_scalar(
                mask_b[:], iot_m[:], -_E, None, op0=ALU.is_equal
            )
            nc.vector.tensor_add(mask[:], mask[:], mask_b[:])
            # Pre-scale the chain masks so the combine needs no scaling:
            # s-chain mask = sqrt(0.5)*one-hot -> s'^2 = 0.5*s^2 (up to the
            # bf16 rounding of sqrt(0.5): (c^2-0.5)*s^2 ~ 1e-4 rel, noise
            # next to the 2.4e-3 bf16-squares error); q-chain mask =
            # 0.5*one-hot (exact in bf16) -> q' = 0.5*q.
            maskh = cpool.tile([_P, _E], bf16, tag="maskh")
            maskq = cpool.tile([_P, _E], bf16, tag="maskq")
            nc.vector.tensor_scalar_mul(maskh[:], mask[:], HALF_SQRT)
            nc.vector.tensor_scalar_mul(maskq[:], mask[:], 0.5)

            for c in range(n_chunks):
                rows = slice(c * _P, (c + 1) * _P)
                # separate banks: a start=True matmul clears its whole bank,
                # so the two accumulation chains must not share one
                s_t = accpool.tile([_P, _E], f32, tag="s")
                q_t = accpool.tile([_P, _E], f32, tag="q")
                s_ps = s_t[:]
                q_ps = q_t[:]
                for blk0, nblk in halves:
                    cols = slice(blk0 * _P, (blk0 + nblk) * _P)
                    n = nblk * _P
                    xt = xpool.tile([_P, n], f32, tag="x")
                    nc.sync.dma_start(out=xt[:], in_=x[rows, cols])
                    xbt = xbpool.tile([_P, n], bf16, tag="xb")

                    xT = xtpool.tile([_P, n], bf16, tag="xT")
                    xsq = sqpool.tile([_P, n], bf16, tag="xsq")
                    g0 = 0
                    for gn in groups:
                        gcols = slice(g0 * _P, (g0 + gn) * _P)
                        # per-group cast: lets the tail chunk pipeline at
                        # group granularity (gpsimd CAST measured 4x slower
                        # than DVE's 2x mode, so this stays on DVE)
                        nc.vector.tensor_copy(xbt[:, gcols], xt[:, gcols])
                        pt = ptpool.tile([_P, groups[0] * _P], bf16, tag="pt")
                        for j in range(gn):
                            k = g0 + j
                            nc.tensor.transpose(
                                pt[:, j * _P : (j + 1) * _P],
                                xbt[:, k * _P : (k + 1) * _P],
                                ident[:],
                            )
                        nc.vector.tensor_copy(xT[:, gcols], pt[:, : gn * _P])
                        nc.scalar.activation(xsq[:, gcols], pt[:, : gn * _P], SQ)
                        g0 += gn
                    for k in range(nblk):
                        kk = blk0 + k
                        bcols = slice(k * _P, (k + 1) * _P)
                        nc.tensor.matmul(
                            s_ps,
                            xT[:, bcols],
                            maskh[:],
                            start=(kk == 0),
                            stop=(kk == _NBLK - 1),
                        )
                        nc.tensor.matmul(
                            q_ps,
                            xsq[:, bcols],
                            maskq[:],
                            start=(kk == 0),
                            stop=(kk == _NBLK - 1),
                        )

                # res = s'^2 - q' = 0.5*s^2 - 0.5*q
                m2 = spool.tile([_P, _E], f32, tag="m2")
                res = spool.tile([_P, _E], f32, tag="res")
                nc.scalar.activation(m2[:], s_ps, SQ)
                nc.vector.tensor_sub(res[:], m2[:], q_ps)
                nc.sync.dma_start(out=out[rows, :], in_=res[:])
    nc.compile()
    return nc


_WALRUS_EXTRA = []


def _patch_walrus():
    """Cap walrus's semaphore allocation: the NEFF postamble zeroes every
    allocated semaphore one event-sem op at a time (spaced to dodge the
    event-accel erratum), so unused semaphores cost ~150ns each at the
    kernel tail."""
    from concourse import bass_utils

    if getattr(bass_utils, "_walrus_patched", False):
        return
    real_run = bass_utils.run_command

    def run2(cmd, **kw):
        if cmd and "walrus_driver" in str(cmd[0]):
            cmd = list(cmd) + _WALRUS_EXTRA
        return real_run(cmd, **kw)

    bass_utils.run_command = run2
    bass_utils._walrus_patched = True


def _run(in_maps, **kwargs):
    from concourse.bass_utils import run_bass_kernel_spmd

    _patch_walrus()
    nc = _build()
    return run_bass_kernel_spmd(nc, in_maps, core_ids=list(range(_NCORES)), **kwargs)


def _shard(inputs: np.ndarray):
    x = np.ascontiguousarray(
        np.asarray(inputs, dtype=np.float32).reshape(_B, _ROW)
    )
    return [
        {"inputs": np.ascontiguousarray(x[i * _BS : (i + 1) * _BS])}
        for i in range(_NCORES)
    ]


def kernel(
    inputs: np.ndarray,
    weight_attention: np.ndarray = None,
    weight_projection: np.ndarray = None,
    weight_bias: np.ndarray = None,
) -> np.ndarray:
    # weights are dead code (softmax over a size-1 axis == 1.0)
    res = _run(_shard(inputs))
    return np.concatenate([r["out"] for r in res.results], axis=0)
